# revision 45
# baseline (speedup 1.0000x reference)
"""MoE FFN (top-2 of 8 experts) on 8 Trainium2 NeuronCores, expert-parallel.

Strategy (expert-parallel per the sharding hint, plus hot/cold pairing):
  - Host: router (x @ router_w, softmax, top-2, renormalize) + dispatch:
    gather each expert's tokens into a padded [D, C] block (transposed so
    the device kernel gets D on the partition axis).
  - Experts are paired hot-with-cold; core pair (2j, 2j+1) serves expert
    pair j, each core computing one H-half of BOTH experts (capacity is
    per-segment max, so the hot expert's overflow doesn't pad every core).
  - Device (SPMD, bf16): per segment, y += gelu(x @ w1 + b1) @ w2 as two
    tiled matmul phases; all w1 tiles SBUF-resident, phase 1 chunk-outer so
    only (w1[hc0], x chunk0) gate the start; x/y on the scalar HWDGE ring,
    weights on the sync ring; stationary-weight reuse + LDWEIGHTS dedup.
  - Host: combine partial halves: out[t] = sum_k w[t,k] *
    (y_half0[t] + y_half1[t] + b2[e_k]).

Self-contained: shapes hardcoded for B=2, S=2048, D=1024, H=4096, E=8, top-2.
"""

import math
import os
from contextlib import ExitStack

import ml_dtypes
import numpy as np

import concourse.bass as bass
import concourse.tile as tile
from concourse import bacc, mybir
from concourse._compat import with_exitstack
from concourse.bass_utils import run_bass_kernel_spmd

B, S, D, H, E, TOP_K = 2, 2048, 1024, 4096, 8, 2
T = B * S
P = 128
N_CORES = 8
KC = D // P   # 8  k-chunks of the d contraction
HC = H // P   # 32 chunks of the hidden dim
DC = D // P   # 8  chunks of the output dim

_DT = {"bf16": mybir.dt.bfloat16, "fp32r": mybir.dt.float32r,
       "fp32": mybir.dt.float32}
_NPDT = {"bf16": np.dtype(ml_dtypes.bfloat16), "fp32r": np.dtype(np.float32),
         "fp32": np.dtype(np.float32)}

# matmul precision: "fp32" (exact, 1/4 PE rate), "fp32r" (TF32-like, full
# rate), "bf16" (full rate, halves weight DMA traffic).
# Default: bf16, single pass per segment, 512-capped token chunks (PSUM
# bank limit), x/y traffic on the scalar HWDGE ring so it overlaps the
# weight stream on the sync ring, x loaded in kc-half pieces so the first
# matmuls start ~4us in instead of waiting for the full x block.
DEFAULT_CFG = dict(
    mode=os.environ.get("MOE_DTYPE", "bf16"),
    chunk=512,    # max moving-operand columns per matmul (PSUM bank limit)
    cgran=32,     # capacity rounding granularity
    ps1=4, ps2=4,  # PSUM pool bufs for phase 1 / phase 2 (0 ps1 = shared pool)
    w1b=0, w2b=3, xb=1, yb=2, hb=1,  # w1b 0 = auto (1 if w1-resident else 2)
    passw=1792,   # 0 = auto from SBUF budget
    w2split=False,  # stream w2 as two half tiles (frees 8KB/partition SBUF)
    wreuse=False,  # keep stationary weights across token chunks (kc-outer)
    ldwopt=False,  # pass --enable-ldw-opt=true to walrus
    dedup=True,    # remove consecutive duplicate InstLdweights
    xq="scalar",  # engine ring for x-in/y-out DMA ("scalar" | "sync")
    xsplit=2,     # x load granularity: 0=whole, 1=per chunk, 2=per (chunk,kc)
    p1reuse=True, p2reuse=True,  # stationary-weight reuse orderings
    halfh=True,   # pair hot/cold experts; each core computes one H-half of
                  # both (better load balance: capacity per pair, not max)
    ybf16=False,  # DMA y partials out in bf16 (halves output traffic)
)

LAST_RESULTS = None  # BassKernelResults of the most recent device run
_CACHE: dict = {}

# When True, the walrus invocation gets --enable-ldw-opt=true so redundant
# LDWEIGHTS (same stationary operand across consecutive matmuls, see
# wreuse) are removed from the PE stream.
LDWOPT_ACTIVE = False
_orig_run_command = None


def _install_ldwopt_patch():
    global _orig_run_command
    if _orig_run_command is not None:
        return
    from concourse import bass_utils as _bu
    _orig_run_command = _bu.run_command

    def patched(argv, **kwargs):
        if LDWOPT_ACTIVE:
            argv = ["--enable-ldw-opt=true" if a == "--enable-ldw-opt=false"
                    else a for a in argv]
        return _orig_run_command(argv, **kwargs)

    _bu.run_command = patched


def _chunk_outer(cfg):
    # phase-1 chunk-outer order keeps all HC w1 tiles SBUF-resident; only
    # fits at 2-byte dtypes
    co = cfg.get("p1order", "auto")
    if co == "auto":
        return cfg["mode"] == "bf16" and not cfg["wreuse"]
    return co == "chunk_outer"


def _chunks(width, chunk):
    # near-equal chunks of <= chunk columns (equal widths beat a ragged tail);
    # widths kept multiples of 8 (odd moving-dim fp32r matmuls fail codegen)
    k = (width + chunk - 1) // chunk
    base = min(chunk, -(-(width // k) // 8) * 8)
    out = []
    o = 0
    while o < width:
        c = min(base, width - o)
        out.append((o, c))
        o += c
    return out


def _passes(C, cfg):
    # h (the [H, pass_w] hidden activations) stays SBUF-resident per pass:
    # pass_w*HC*esize bytes/partition.
    max_w = cfg["passw"] or (1792 if cfg["mode"] == "bf16" else 768)
    n = max(1, math.ceil(C / max_w))
    return _chunks(C, math.ceil(C / n))


@with_exitstack
def _ffn_body(ctx: ExitStack, tc: tile.TileContext, segments, cfg, reps=1):
    """segments: list of (seg_tag, yT, xgT, w1, b1, w2, C, hcnt)."""
    nc = tc.nc

    singles = ctx.enter_context(tc.tile_pool(name="singles", bufs=1))
    xpool = ctx.enter_context(tc.tile_pool(name="xg", bufs=cfg["xb"]))
    w1b = cfg["w1b"] or (1 if _chunk_outer(cfg) else 2)
    w1pool = ctx.enter_context(tc.tile_pool(name="w1", bufs=w1b))
    w2pool = ctx.enter_context(tc.tile_pool(name="w2", bufs=cfg["w2b"]))
    hpool = ctx.enter_context(tc.tile_pool(name="h", bufs=cfg["hb"]))
    ypool = ctx.enter_context(tc.tile_pool(name="y", bufs=cfg["yb"]))
    if cfg["ps1"]:
        ps1pool = ctx.enter_context(
            tc.tile_pool(name="ps1", bufs=cfg["ps1"], space="PSUM"))
        ps2pool = ctx.enter_context(
            tc.tile_pool(name="ps2", bufs=cfg["ps2"], space="PSUM"))
    else:  # shared single pool sized by ps2
        ps1pool = ps2pool = ctx.enter_context(
            tc.tile_pool(name="ps", bufs=cfg["ps2"], space="PSUM"))

    xeng = nc.scalar if cfg.get("xq") == "scalar" else nc.sync
    b1_sbs = []
    for (seg, yT, xgT, w1, b1, w2, C, hcnt) in segments:
        b1_sb = singles.tile([P, hcnt], mybir.dt.float32, name=f"b1_{seg}")
        xeng.dma_start(b1_sb, b1)
        b1_sbs.append(b1_sb)

    def run_all(u=0):
        # u>0 reuses the same tile names: buffer-reuse serialization matches
        # the For_i iteration boundary, without the loop-control overhead
        for i, (seg, yT, xgT, w1, b1, w2, C, hcnt) in enumerate(segments):
            _do_passes(tc, C, cfg, b1_sbs[i], xgT, w1, w2, yT,
                       xpool, w1pool, w2pool, hpool, ypool, ps1pool, ps2pool,
                       hcnt=hcnt, seg=seg, first_seg=(i == 0 and u == 0))

    if reps == 1:
        run_all()
    else:
        with tc.For_i(0, reps, 1):
            for u in range(cfg.get("unroll", 1)):
                run_all(u)


def _do_passes(tc, C, cfg, b1_sb, xgT, w1, w2, yT,
               xpool, w1pool, w2pool, hpool, ypool, ps1pool, ps2pool,
               hcnt=HC, seg="s", first_seg=True):
    nc = tc.nc
    in_dt = _DT[cfg["mode"]]
    h_dt = in_dt
    chunk = cfg["chunk"] if cfg["mode"] == "bf16" else min(cfg["chunk"], 512)
    xeng = nc.scalar if cfg.get("xq") == "scalar" else nc.sync

    # host pre-arranges weights into SBUF tile layout:
    #   w1: [P, hcnt, KC, 128]  (w1_dev[p, hc, kc, h] = w1[kc*128+p, hc*128+h])
    #   w2: [P, DC, hcnt, 128]  (w2_dev[p, dc, hc, d] = w2[hc*128+p, dc*128+d])
    # so each per-tile DMA reads 2-8KB contiguous per partition line.
    xgT_r = xgT.rearrange("(kc p) c -> p kc c", p=P)
    yT_r = yT.rearrange("(dc p) c -> dc p c", p=P)

    chunk_outer = _chunk_outer(cfg)

    first_pass = first_seg
    for (poff, pw) in _passes(C, cfg):
        chunks = _chunks(pw, chunk)
        if cfg.get("maxchunks"):
            chunks = chunks[:cfg["maxchunks"]]
        xg = xpool.tile([P, KC, pw], in_dt, name=f"xg_{seg}")
        w1t0 = None
        kh = KC // 2
        if first_pass and cfg.get("xsplit", 0):
            # critical start path, in consumption order: w1[hc0] tile, then
            # x chunk0 in two kc-halves (the first 4 matmuls unblock after
            # the first half). Later chunks are issued after the w1 DMA
            # stream (not needed until their turn in the chunk-outer order).
            first_pass = False
            with tc.high_priority():
                w1t0 = w1pool.tile([P, KC, P], in_dt,
                                   name=f"w1t_{seg}_{poff}_0")
                nc.sync.dma_start(w1t0, w1[:, 0, :, :])
                (o0, cw0) = chunks[0]
                xeng.dma_start(xg[:, :kh, o0:o0 + cw0],
                               xgT_r[:, :kh, poff + o0:poff + o0 + cw0])
                xeng.dma_start(xg[:, kh:, o0:o0 + cw0],
                               xgT_r[:, kh:, poff + o0:poff + o0 + cw0])
            xrest = chunks[1:]
        elif cfg.get("xsplit", 0):
            xrest = chunks
        else:
            xeng.dma_start(xg, xgT_r[:, :, poff:poff + pw])
            xrest = []
        h = hpool.tile([P, hcnt, pw], h_dt, name=f"h_{seg}")
        tag1 = "ps1" if ps1pool is not ps2pool else "ps"
        tag2 = "ps2" if ps1pool is not ps2pool else "ps"

        if chunk_outer:
            # phase 1, chunk-outer: all w1 tiles stay resident; x chunk c
            # is consumed by hcnt*KC matmuls before chunk c+1 is touched, so
            # only (w1[hc0], x chunk0) gate the start
            w1ts = [w1t0] if w1t0 is not None else []
            for hc in range(len(w1ts), hcnt):
                w1t = w1pool.tile([P, KC, P], in_dt,
                                  name=f"w1t_{seg}_{poff}_{hc}")
                nc.sync.dma_start(w1t, w1[:, hc, :, :])
                w1ts.append(w1t)
            for (o, cw) in xrest:
                xeng.dma_start(xg[:, :kh, o:o + cw],
                               xgT_r[:, :kh, poff + o:poff + o + cw])
                xeng.dma_start(xg[:, kh:, o:o + cw],
                               xgT_r[:, kh:, poff + o:poff + o + cw])
            ci = 0
            while ci < len(chunks):
                if cfg.get("p1reuse") and ci > 0:
                    # remaining chunks together, kc-outer: one stationary
                    # load serves len(rest) matmuls (fewer LDWEIGHTS)
                    rest = chunks[ci:]
                    for hc in range(hcnt):
                        pss = [ps1pool.tile([P, chunk], mybir.dt.float32,
                                            tag=tag1, name=f"ps1r_{cj}")
                               for cj in range(len(rest))]
                        for kc in range(KC):
                            for cj, (o, cw) in enumerate(rest):
                                nc.tensor.matmul(
                                    pss[cj][:, :cw], w1ts[hc][:, kc, :],
                                    xg[:, kc, o:o + cw],
                                    start=(kc == 0), stop=(kc == KC - 1))
                        for cj, (o, cw) in enumerate(rest):
                            nc.scalar.activation(
                                h[:, hc, o:o + cw], pss[cj][:, :cw],
                                mybir.ActivationFunctionType.Gelu,
                                bias=b1_sb[:, hc:hc + 1], scale=1.0)
                    break
                (o, cw) = chunks[ci]
                for hc in range(hcnt):
                    ps = ps1pool.tile([P, chunk], mybir.dt.float32, tag=tag1)
                    for kc in range(KC):
                        nc.tensor.matmul(
                            ps[:, :cw], w1ts[hc][:, kc, :], xg[:, kc, o:o + cw],
                            start=(kc == 0), stop=(kc == KC - 1))
                    if not cfg.get("skip_act"):
                        nc.scalar.activation(
                            h[:, hc, o:o + cw], ps[:, :cw],
                            mybir.ActivationFunctionType.Gelu,
                            bias=b1_sb[:, hc:hc + 1], scale=1.0)
                ci += 1
            if cfg.get("skip_p2"):
                yt = ypool.tile([P, 8], mybir.dt.float32, tag="yt",
                                name=f"ytp1_{seg}")
                src = xg[:, 0, 0:8] if cfg.get("skip_act") else h[:, 0, 0:8]
                nc.vector.tensor_copy(yt, src)
                xeng.dma_start(yT_r[0, :, poff:poff + 8], yt)
            else:
                _phase2(tc, cfg, chunks, chunk, h, w2, yT_r, poff,
                        w2pool, ypool, ps2pool, tag2, in_dt, xeng,
                        hcnt=hcnt, seg=seg)
            continue

        for (o, cw) in xrest:
            xeng.dma_start(xg[:, :, o:o + cw],
                           xgT_r[:, :, poff + o:poff + o + cw])

        # phase 1: h = gelu(x @ w1 + b1), computed as hT[hc, t] chunks
        for hc in range(HC):
            if hc == 0 and w1t0 is not None:
                w1t = w1t0
            else:
                w1t = w1pool.tile([P, KC, P], in_dt)
                nc.sync.dma_start(w1t, w1[:, hc, :, :])
            if cfg["wreuse"]:
                # kc-outer: one stationary load serves every token chunk
                pss = [ps1pool.tile([P, chunk], mybir.dt.float32, tag=tag1,
                                    name=f"ps1_{hc}_{ci}")
                       for ci in range(len(chunks))]
                for kc in range(KC):
                    for ci, (o, cw) in enumerate(chunks):
                        nc.tensor.matmul(
                            pss[ci][:, :cw], w1t[:, kc, :], xg[:, kc, o:o + cw],
                            start=(kc == 0), stop=(kc == KC - 1))
                for ci, (o, cw) in enumerate(chunks):
                    nc.scalar.activation(
                        h[:, hc, o:o + cw], pss[ci][:, :cw],
                        mybir.ActivationFunctionType.Gelu,
                        bias=b1_sb[:, hc:hc + 1], scale=1.0)
            else:
                for (o, cw) in chunks:
                    ps = ps1pool.tile([P, chunk], mybir.dt.float32, tag=tag1)
                    for kc in range(KC):
                        nc.tensor.matmul(
                            ps[:, :cw], w1t[:, kc, :], xg[:, kc, o:o + cw],
                            start=(kc == 0), stop=(kc == KC - 1))
                    nc.scalar.activation(
                        h[:, hc, o:o + cw], ps[:, :cw],
                        mybir.ActivationFunctionType.Gelu,
                        bias=b1_sb[:, hc:hc + 1], scale=1.0)

        # phase 2: yT[dc, t] = (h.T @ w2) chunks
        for dc in range(DC):
            halves = None
            if cfg.get("w2split") and not cfg["wreuse"]:
                # half-size w2 tiles free 8KB/partition of SBUF (lets the
                # whole hidden activation block fit in a single pass)
                hh = HC // 2
                w2ta = w2pool.tile([P, hh, P], in_dt, tag="w2t",
                                   name=f"w2a_{poff}_{dc}")
                w2eng.dma_start(w2ta, w2[:, dc, :hh, :])
                w2tb = w2pool.tile([P, hh, P], in_dt, tag="w2t",
                                   name=f"w2b_{poff}_{dc}")
                w2eng.dma_start(w2tb, w2[:, dc, hh:, :])
                halves = [(w2ta, 0), (w2tb, hh)]
                w2t = None
            else:
                w2t = w2pool.tile([P, HC, P], in_dt, tag="w2t",
                                  name=f"w2t_{poff}_{dc}")
                nc.sync.dma_start(w2t, w2[:, dc, :, :])
            if cfg["wreuse"]:
                pss = [ps2pool.tile([P, chunk], mybir.dt.float32, tag=tag2,
                                    name=f"ps2_{dc}_{ci}")
                       for ci in range(len(chunks))]
                for hc in range(HC):
                    for ci, (o, cw) in enumerate(chunks):
                        nc.tensor.matmul(
                            pss[ci][:, :cw], w2t[:, hc, :], h[:, hc, o:o + cw],
                            start=(hc == 0), stop=(hc == HC - 1))
                for ci, (o, cw) in enumerate(chunks):
                    yt = ypool.tile([P, cw], mybir.dt.float32, tag="yt",
                                    name=f"yt_{dc}_{ci}")
                    nc.vector.tensor_copy(yt[:, :cw], pss[ci][:, :cw])
                    xeng.dma_start(
                        yT_r[dc, :, poff + o:poff + o + cw], yt[:, :cw])
            else:
                dchunks = chunks
                if dc == DC - 1 and cfg.get("tailsplit", True):
                    # halve the final chunk so its copy+DMA-out overlaps the
                    # matmuls of the other half (shortens the drain tail)
                    (o_l, cw_l) = chunks[-1]
                    ch = -(-(cw_l // 2) // 8) * 8
                    dchunks = list(chunks[:-1]) + [(o_l, ch),
                                                   (o_l + ch, cw_l - ch)]
                for (o, cw) in dchunks:
                    ps2 = ps2pool.tile([P, chunk], mybir.dt.float32, tag=tag2)
                    for hc in range(HC):
                        if halves is None:
                            lhsT = w2t[:, hc, :]
                        else:
                            wt, base = halves[hc // (HC // 2)]
                            lhsT = wt[:, hc - base, :]
                        nc.tensor.matmul(
                            ps2[:, :cw], lhsT, h[:, hc, o:o + cw],
                            start=(hc == 0), stop=(hc == HC - 1))
                    yt = ypool.tile([P, cw], mybir.dt.float32, tag="yt",
                                    name=f"yt_{dc}_{o}")
                    nc.vector.tensor_copy(yt[:, :cw], ps2[:, :cw])
                    xeng.dma_start(
                        yT_r[dc, :, poff + o:poff + o + cw], yt[:, :cw])


def _phase2(tc, cfg, chunks, chunk, h, w2, yT_r, poff,
            w2pool, ypool, ps2pool, tag2, in_dt, xeng, hcnt=HC, seg="s"):
    nc = tc.nc
    y_dt = in_dt if cfg.get("ybf16") else mybir.dt.float32
    w2eng = xeng if cfg.get("w2q") == "scalar" else nc.sync
    for dc in range(DC):
        halves = None
        if cfg.get("w2split"):
            hh = hcnt // 2
            w2ta = w2pool.tile([P, hh, P], in_dt, tag="w2t",
                               name=f"w2a_{seg}_{poff}_{dc}")
            w2eng.dma_start(w2ta, w2[:, dc, :hh, :])
            w2tb = w2pool.tile([P, hh, P], in_dt, tag="w2t",
                               name=f"w2b_{seg}_{poff}_{dc}")
            w2eng.dma_start(w2tb, w2[:, dc, hh:, :])
            halves = [(w2ta, 0), (w2tb, hh)]
            w2t = None
        else:
            w2t = w2pool.tile([P, hcnt, P], in_dt, tag="w2t",
                              name=f"w2t_{seg}_{poff}_{dc}")
            w2eng.dma_start(w2t, w2[:, dc, :, :])
        dchunks = chunks
        if dc == DC - 1 and cfg.get("tailsplit", True):
            (o_l, cw_l) = chunks[-1]
            ch = -(-(cw_l // 2) // 8) * 8
            dchunks = list(chunks[:-1]) + [(o_l, ch), (o_l + ch, cw_l - ch)]

        def lhs(hc):
            if halves is None:
                return w2t[:, hc, :]
            wt, base = halves[hc // (hcnt // 2)]
            return wt[:, hc - base, :]

        if cfg.get("p2reuse") and dc < DC - 1:
            # hc-outer: one stationary load serves every token chunk
            pss = [ps2pool.tile([P, chunk], mybir.dt.float32, tag=tag2,
                                name=f"ps2r_{cj}")
                   for cj in range(len(dchunks))]
            for hc in range(hcnt):
                for cj, (o, cw) in enumerate(dchunks):
                    nc.tensor.matmul(
                        pss[cj][:, :cw], lhs(hc), h[:, hc, o:o + cw],
                        start=(hc == 0), stop=(hc == hcnt - 1))
            for cj, (o, cw) in enumerate(dchunks):
                yt = ypool.tile([P, cw], y_dt, tag="yt",
                                name=f"yt_{seg}_{dc}_{o}")
                nc.vector.tensor_copy(yt[:, :cw], pss[cj][:, :cw])
                xeng.dma_start(
                    yT_r[dc, :, poff + o:poff + o + cw], yt[:, :cw])
            continue
        for (o, cw) in dchunks:
            ps2 = ps2pool.tile([P, chunk], mybir.dt.float32, tag=tag2)
            for hc in range(hcnt):
                nc.tensor.matmul(
                    ps2[:, :cw], lhs(hc), h[:, hc, o:o + cw],
                    start=(hc == 0), stop=(hc == hcnt - 1))
            yt = ypool.tile([P, cw], y_dt, tag="yt",
                            name=f"yt_{seg}_{dc}_{o}")
            nc.vector.tensor_copy(yt[:, :cw], ps2[:, :cw])
            xeng.dma_start(
                yT_r[dc, :, poff + o:poff + o + cw], yt[:, :cw])


def _fold_mm_incs(nc):
    """Drop the per-matmul semaphore increment from non-stop matmuls (only
    the accumulation group's stop-MM keeps its inc) and remap every wait
    threshold on those semaphores to the new counting. MMs complete in PE
    program order, so a waiter on a mid-group count now unblocks at the
    group's stop-MM — always later, never earlier. Saves the ~26ns
    serialized EVT_SEM write per matmul."""
    import bass_rust
    from concourse import mybir as mb

    folded = 0
    for blk in nc.m.functions[0].blocks:
        mm_sems = set()
        for inst in blk.instructions:
            si = inst.sync_info
            if isinstance(inst, mb.InstMatmult) and si:
                for u in si.on_update:
                    if u.sync_type == "semaphore" and \
                            u.update_mode == "sem-inc" and \
                            u.update_reg is None and u.update_value == 1:
                        mm_sems.add(u.id)
        for inst in blk.instructions:
            si = inst.sync_info
            if si is None:
                continue
            if not isinstance(inst, mb.InstMatmult):
                for u in si.on_update:
                    mm_sems.discard(u.id)
            for w in si.on_wait:
                # only immediate GE waits are remappable
                if w.id in mm_sems and w.wait_mode != "sem-ge-imm":
                    mm_sems.discard(w.id)
        if not mm_sems:
            continue

        # per sem: old cumulative position -> new cumulative after remap,
        # where an old threshold T maps to the count of kept incs up to the
        # first kept inc at position >= T
        mms = [i for i in blk.instructions if isinstance(i, mb.InstMatmult)]
        remap = {}
        for sid in mm_sems:
            keeps = []
            for j, inst in enumerate(mms):
                si = inst.sync_info
                if not si or not any(u.id == sid for u in si.on_update):
                    continue
                keeps.append(inst.stop_tensor_calc or inst is mms[-1])
            total_old = len(keeps)
            fwd = [0] * (total_old + 1)  # fwd[T] for T in 1..total
            kept_cum = 0
            pending = []
            for pos in range(1, total_old + 1):
                pending.append(pos)
                if keeps[pos - 1]:
                    kept_cum += 1
                    for p in pending:
                        fwd[p] = kept_cum
                    pending = []
            for p in pending:  # trailing dropped (shouldn't happen)
                fwd[p] = kept_cum
            remap[sid] = fwd

        # bail on sems with waits beyond this block's total (cross-block
        # counting would make the per-block remap unsound)
        for inst in blk.instructions:
            si = inst.sync_info
            if si:
                for w in si.on_wait:
                    if w.id in remap and w.wait_value >= len(remap[w.id]):
                        del remap[w.id]
        mm_sems &= set(remap)
        if not mm_sems:
            continue

        # rewrite updates on MMs
        for sid in mm_sems:
            seen = 0
            for inst in mms:
                si = inst.sync_info
                if not si or not any(u.id == sid for u in si.on_update):
                    continue
                seen += 1
                keep = inst.stop_tensor_calc or inst is mms[-1]
                if not keep:
                    new_upd = [u for u in si.on_update if u.id != sid]
                    inst.sync_info = bass_rust.SyncInfo(
                        on_wait=list(si.on_wait), on_update=new_upd)
                    folded += 1

        # rewrite waits everywhere in the block
        for inst in blk.instructions:
            si = inst.sync_info
            if not si or not si.on_wait:
                continue
            new_w, changed = [], False
            for w in si.on_wait:
                if w.id in mm_sems:
                    fwd = remap[w.id]
                    t = min(max(w.wait_value, 0), len(fwd) - 1)
                    nv = fwd[t] if t > 0 else 0
                    if nv != w.wait_value:
                        w = bass_rust.SyncWait(
                            sync_type=w.sync_type, id=w.id,
                            ant_name=w.ant_name, wait_mode=w.wait_mode,
                            wait_value=nv, wait_reg=None)
                        changed = True
                new_w.append(w)
            if changed:
                inst.sync_info = bass_rust.SyncInfo(
                    on_wait=new_w, on_update=list(si.on_update))
    return folded


def _dedupe_ldweights(nc):
    """Drop an InstLdweights that reloads exactly what the PE already holds
    (same SBUF slot/offset/pattern, no semaphores attached). Safe because the
    wreuse ordering makes duplicates strictly consecutive in PE order."""
    def sig(ap):
        return (ap.memref, ap.offset, str(ap.ap), ap.dtype)

    removed = 0
    for blk in nc.m.functions[0].blocks:
        out = []
        last_sig = None
        changed = False
        for inst in blk.instructions:
            if inst.engine == mybir.EngineType.PE:
                if isinstance(inst, mybir.InstLdweights):
                    s = sig(inst.ins[0])
                    si = inst.sync_info
                    no_sync = (si is None) or (
                        not si.on_wait and not si.on_update)
                    if s == last_sig and no_sync:
                        removed += 1
                        changed = True
                        continue
                    last_sig = s
                elif not isinstance(inst, mybir.InstMatmult):
                    last_sig = None  # drains/branches etc: invalidate
            out.append(inst)
        if changed:
            blk.instructions = out
    return removed


def _build(C, cfg, reps=1):
    key = (C, reps, tuple(sorted(cfg.items())))
    if key in _CACHE:
        return _CACHE[key]
    dt_in = _DT[cfg["mode"]]
    nc = bacc.Bacc("TRN2", target_bir_lowering=False, debug=False,
                   num_devices=N_CORES)
    if cfg.get("halfh"):
        Ca, Cb = C
        hh = HC // 2
        segs = []
        for seg, Cs in (("a", Ca), ("b", Cb)):
            xgT = nc.dram_tensor(f"xgT_{seg}", (D, Cs), dt_in,
                                 kind="ExternalInput").ap()
            w1a = nc.dram_tensor(f"w1_{seg}", (P, hh, KC, P), dt_in,
                                 kind="ExternalInput").ap()
            b1a = nc.dram_tensor(f"b1_{seg}", (P, hh), mybir.dt.float32,
                                 kind="ExternalInput").ap()
            w2a = nc.dram_tensor(f"w2_{seg}", (P, DC, hh, P), dt_in,
                                 kind="ExternalInput").ap()
            y_dt = dt_in if cfg.get("ybf16") else mybir.dt.float32
            yT = nc.dram_tensor(f"yT_{seg}", (D, Cs), y_dt,
                                kind="ExternalOutput").ap()
            segs.append((seg, yT, xgT, w1a, b1a, w2a, Cs, hh))
    else:
        xgT = nc.dram_tensor("xgT", (D, C), dt_in, kind="ExternalInput").ap()
        w1a = nc.dram_tensor("w1", (P, HC, KC, P), dt_in,
                             kind="ExternalInput").ap()
        b1a = nc.dram_tensor("b1", (P, HC), mybir.dt.float32,
                             kind="ExternalInput").ap()
        w2a = nc.dram_tensor("w2", (P, DC, HC, P), dt_in,
                             kind="ExternalInput").ap()
        y_dt = dt_in if cfg.get("ybf16") else mybir.dt.float32
        yT = nc.dram_tensor("yT", (D, C), y_dt,
                            kind="ExternalOutput").ap()
        segs = [("s", yT, xgT, w1a, b1a, w2a, C, HC)]
    with tile.TileContext(nc) as tc:
        _ffn_body(tc, segs, cfg, reps=reps)
    nc.compile()
    if cfg["dedup"]:
        _dedupe_ldweights(nc)
    _CACHE[key] = nc
    return nc


def _w1_dev(w1_e, np_in, hcnt=HC):
    # [D, h] -> [P, hcnt, KC, 128]: w1_dev[p, hc, kc, h] = w1[kc*128+p, hc*128+h]
    return np.ascontiguousarray(
        w1_e.reshape(KC, P, hcnt, P).transpose(1, 2, 0, 3)).astype(np_in)


def _w2_dev(w2_e, np_in, hcnt=HC):
    # [h, D] -> [P, DC, hcnt, 128]: w2_dev[p, dc, hc, d] = w2[hc*128+p, dc*128+d]
    return np.ascontiguousarray(
        w2_e.reshape(hcnt, P, DC, P).transpose(1, 2, 0, 3)).astype(np_in)


def _route(xf, router_w):
    """Replicates the reference router in fp32 numpy: softmax, top-2,
    renormalize. Returns per-expert token ids and combine weights."""
    logits = xf @ np.asarray(router_w, dtype=np.float32)          # [T, E]
    m = logits.max(axis=-1, keepdims=True)
    z = np.exp(logits - m)
    probs = z / z.sum(axis=-1, keepdims=True)
    idx = np.argpartition(-probs, TOP_K - 1, axis=-1)[:, :TOP_K]  # [T, 2]
    vals = np.take_along_axis(probs, idx, axis=-1)
    wn = vals / vals.sum(axis=-1, keepdims=True)

    eflat = idx.reshape(-1)
    tflat = np.repeat(np.arange(T), TOP_K)
    wflat = wn.reshape(-1).astype(np.float32)
    order = np.argsort(eflat, kind="stable")
    counts = np.bincount(eflat, minlength=E)
    starts = np.concatenate([[0], np.cumsum(counts)])
    toks, wts = [], []
    for e in range(E):
        sel = order[starts[e]:starts[e + 1]]
        toks.append(tflat[sel])
        wts.append(wflat[sel])
    return toks, wts, counts


def _capacity(counts, cfg):
    g = cfg["cgran"]
    if cfg.get("halfh"):
        pairs = _pairing(counts)

        def cap(cs):
            return max(cfg["chunk"],
                       int(math.ceil(max(cs) / g)) * g)
        return (cap([counts[a] for a, _ in pairs]),
                cap([counts[b] for _, b in pairs]))
    return max(cfg["chunk"], int(math.ceil(counts.max() / g)) * g)


def _pairing(counts):
    # hot expert paired with cold: core pair j serves experts pairs[j];
    # core 2j holds the lower H-half of both, core 2j+1 the upper half
    order = np.argsort(-np.asarray(counts), kind="stable")
    return [(int(order[i]), int(order[E - 1 - i])) for i in range(E // 2)]


def _in_maps(xf, toks, counts, C, w1, b1, w2, cfg):
    np_in = _NPDT[cfg["mode"]]
    if cfg.get("halfh"):
        Ca, Cb = C
        hh = HC // 2
        pairs = _pairing(counts)
        maps = []
        for (ea, eb) in pairs:
            xga = np.zeros((D, Ca), dtype=np_in)
            xga[:, :counts[ea]] = xf[toks[ea]].T.astype(np_in)
            xgb = np.zeros((D, Cb), dtype=np_in)
            xgb[:, :counts[eb]] = xf[toks[eb]].T.astype(np_in)
            for half in range(2):
                sl = slice(half * (H // 2), (half + 1) * (H // 2))
                maps.append({
                    "xgT_a": xga, "xgT_b": xgb,
                    "w1_a": _w1_dev(w1[ea][:, sl], np_in, hh),
                    "w1_b": _w1_dev(w1[eb][:, sl], np_in, hh),
                    "b1_a": np.ascontiguousarray(
                        b1[ea][sl].reshape(hh, P).T),
                    "b1_b": np.ascontiguousarray(
                        b1[eb][sl].reshape(hh, P).T),
                    "w2_a": _w2_dev(w2[ea][sl, :], np_in, hh),
                    "w2_b": _w2_dev(w2[eb][sl, :], np_in, hh),
                })
        return maps
    maps = []
    for e in range(E):
        ce = counts[e]
        xg = np.zeros((D, C), dtype=np_in)
        xg[:, :ce] = xf[toks[e]].T.astype(np_in)
        maps.append({
            "xgT": xg,
            "w1": _w1_dev(w1[e], np_in),
            "b1": np.ascontiguousarray(b1[e].reshape(HC, P).T),
            "w2": _w2_dev(w2[e], np_in),
        })
    return maps


def kernel(x, router_w, w1, b1, w2, b2):
    global LAST_RESULTS
    x = np.asarray(x, dtype=np.float32)
    w1 = np.asarray(w1, dtype=np.float32)
    b1 = np.asarray(b1, dtype=np.float32)
    w2 = np.asarray(w2, dtype=np.float32)
    b2 = np.asarray(b2, dtype=np.float32)
    cfg = dict(DEFAULT_CFG)

    xf = x.reshape(T, D)
    toks, wts, counts = _route(xf, router_w)
    C = _capacity(counts, cfg)
    if not cfg.get("halfh") and C > 1100 and cfg["mode"] != "bf16":
        # single-pass fp32 h block no longer fits in SBUF; fall back to two
        # overlapped passes (correct, slightly slower)
        cfg["passw"] = (C + 1) // 2
        cfg["hb"] = 2 if C <= 1152 else 1

    nc = _build(C, cfg)
    in_maps = _in_maps(xf, toks, counts, C, w1, b1, w2, cfg)

    global LDWOPT_ACTIVE
    LDWOPT_ACTIVE = bool(cfg["ldwopt"])
    if LDWOPT_ACTIVE:
        _install_ldwopt_patch()
    res = run_bass_kernel_spmd(nc, in_maps, core_ids=list(range(N_CORES)))
    LAST_RESULTS = res

    out = np.zeros((T, D), dtype=np.float32)
    if cfg.get("halfh"):
        for j, (ea, eb) in enumerate(_pairing(counts)):
            r0, r1 = res.results[2 * j], res.results[2 * j + 1]
            for seg, e in (("a", ea), ("b", eb)):
                ce = counts[e]
                if ce == 0:
                    continue
                y = (r0[f"yT_{seg}"][:, :ce].astype(np.float32)
                     + r1[f"yT_{seg}"][:, :ce].astype(np.float32)).T
                out[toks[e]] += wts[e][:, None] * (y + b2[e][None, :])
        return out.reshape(B, S, D)
    for e in range(E):
        ce = counts[e]
        if ce == 0:
            continue
        y = res.results[e]["yT"][:, :ce].astype(np.float32).T  # [ce, D]
        out[toks[e]] += wts[e][:, None] * (y + b2[e][None, :])
    return out.reshape(B, S, D)



# revision 47
# speedup vs baseline: 1.0095x; 1.0095x over previous
"""MoE FFN (top-2 of 8 experts) on 8 Trainium2 NeuronCores, expert-parallel.

Strategy (expert-parallel per the sharding hint, plus hot/cold pairing):
  - Host: router (x @ router_w, softmax, top-2, renormalize) + dispatch:
    gather each expert's tokens into a padded [D, C] block (transposed so
    the device kernel gets D on the partition axis).
  - Experts are paired hot-with-cold; core pair (2j, 2j+1) serves expert
    pair j, each core computing one H-half of BOTH experts (capacity is
    per-segment max, so the hot expert's overflow doesn't pad every core).
  - Device (SPMD, bf16): per segment, y += gelu(x @ w1 + b1) @ w2 as two
    tiled matmul phases; all w1 tiles SBUF-resident, phase 1 chunk-outer so
    only (w1[hc0], x chunk0) gate the start; x/y on the scalar HWDGE ring,
    weights on the sync ring; stationary-weight reuse + LDWEIGHTS dedup.
  - Host: combine partial halves: out[t] = sum_k w[t,k] *
    (y_half0[t] + y_half1[t] + b2[e_k]).

Self-contained: shapes hardcoded for B=2, S=2048, D=1024, H=4096, E=8, top-2.
"""

import math
import os
from contextlib import ExitStack

import ml_dtypes
import numpy as np

import concourse.bass as bass
import concourse.tile as tile
from concourse import bacc, mybir
from concourse._compat import with_exitstack
from concourse.bass_utils import run_bass_kernel_spmd

B, S, D, H, E, TOP_K = 2, 2048, 1024, 4096, 8, 2
T = B * S
P = 128
N_CORES = 8
KC = D // P   # 8  k-chunks of the d contraction
HC = H // P   # 32 chunks of the hidden dim
DC = D // P   # 8  chunks of the output dim

_DT = {"bf16": mybir.dt.bfloat16, "fp32r": mybir.dt.float32r,
       "fp32": mybir.dt.float32}
_NPDT = {"bf16": np.dtype(ml_dtypes.bfloat16), "fp32r": np.dtype(np.float32),
         "fp32": np.dtype(np.float32)}

# matmul precision: "fp32" (exact, 1/4 PE rate), "fp32r" (TF32-like, full
# rate), "bf16" (full rate, halves weight DMA traffic).
# Default: bf16, single pass per segment, 512-capped token chunks (PSUM
# bank limit), x/y traffic on the scalar HWDGE ring so it overlaps the
# weight stream on the sync ring, x loaded in kc-half pieces so the first
# matmuls start ~4us in instead of waiting for the full x block.
DEFAULT_CFG = dict(
    mode=os.environ.get("MOE_DTYPE", "bf16"),
    chunk=512,    # max moving-operand columns per matmul (PSUM bank limit)
    cgran=32,     # capacity rounding granularity
    ps1=4, ps2=4,  # PSUM pool bufs for phase 1 / phase 2 (0 ps1 = shared pool)
    w1b=0, w2b=3, xb=1, yb=2, hb=1,  # w1b 0 = auto (1 if w1-resident else 2)
    passw=1792,   # 0 = auto from SBUF budget
    w2split=False,  # stream w2 as two half tiles (frees 8KB/partition SBUF)
    wreuse=False,  # keep stationary weights across token chunks (kc-outer)
    ldwopt=False,  # pass --enable-ldw-opt=true to walrus
    dedup=True,    # remove consecutive duplicate InstLdweights
    xq="scalar",  # engine ring for x-in/y-out DMA ("scalar" | "sync")
    xsplit=2,     # x load granularity: 0=whole, 1=per chunk, 2=per (chunk,kc)
    p1reuse=True, p2reuse=True,  # stationary-weight reuse orderings
    halfh=True,   # pair hot/cold experts; each core computes one H-half of
                  # both (better load balance: capacity per pair, not max)
    ybf16=False,  # DMA y partials out in bf16 (halves output traffic)
    warmup=32,    # dummy matmuls filling the start gap (keeps PE HAM warm)
)

LAST_RESULTS = None  # BassKernelResults of the most recent device run
_CACHE: dict = {}

# When True, the walrus invocation gets --enable-ldw-opt=true so redundant
# LDWEIGHTS (same stationary operand across consecutive matmuls, see
# wreuse) are removed from the PE stream.
LDWOPT_ACTIVE = False
_orig_run_command = None


def _install_ldwopt_patch():
    global _orig_run_command
    if _orig_run_command is not None:
        return
    from concourse import bass_utils as _bu
    _orig_run_command = _bu.run_command

    def patched(argv, **kwargs):
        if LDWOPT_ACTIVE:
            argv = ["--enable-ldw-opt=true" if a == "--enable-ldw-opt=false"
                    else a for a in argv]
        return _orig_run_command(argv, **kwargs)

    _bu.run_command = patched


def _chunk_outer(cfg):
    # phase-1 chunk-outer order keeps all HC w1 tiles SBUF-resident; only
    # fits at 2-byte dtypes
    co = cfg.get("p1order", "auto")
    if co == "auto":
        return cfg["mode"] == "bf16" and not cfg["wreuse"]
    return co == "chunk_outer"


def _chunks(width, chunk):
    # near-equal chunks of <= chunk columns (equal widths beat a ragged tail);
    # widths kept multiples of 8 (odd moving-dim fp32r matmuls fail codegen)
    k = (width + chunk - 1) // chunk
    base = min(chunk, -(-(width // k) // 8) * 8)
    out = []
    o = 0
    while o < width:
        c = min(base, width - o)
        out.append((o, c))
        o += c
    return out


def _passes(C, cfg):
    # h (the [H, pass_w] hidden activations) stays SBUF-resident per pass:
    # pass_w*HC*esize bytes/partition.
    max_w = cfg["passw"] or (1792 if cfg["mode"] == "bf16" else 768)
    n = max(1, math.ceil(C / max_w))
    return _chunks(C, math.ceil(C / n))


@with_exitstack
def _ffn_body(ctx: ExitStack, tc: tile.TileContext, segments, cfg, reps=1):
    """segments: list of (seg_tag, yT, xgT, w1, b1, w2, C, hcnt)."""
    nc = tc.nc

    singles = ctx.enter_context(tc.tile_pool(name="singles", bufs=1))
    xpool = ctx.enter_context(tc.tile_pool(name="xg", bufs=cfg["xb"]))
    w1b = cfg["w1b"] or (1 if _chunk_outer(cfg) else 2)
    w1pool = ctx.enter_context(tc.tile_pool(name="w1", bufs=w1b))
    w2pool = ctx.enter_context(tc.tile_pool(name="w2", bufs=cfg["w2b"]))
    hpool = ctx.enter_context(tc.tile_pool(name="h", bufs=cfg["hb"]))
    ypool = ctx.enter_context(tc.tile_pool(name="y", bufs=cfg["yb"]))
    if cfg["ps1"]:
        ps1pool = ctx.enter_context(
            tc.tile_pool(name="ps1", bufs=cfg["ps1"], space="PSUM"))
        ps2pool = ctx.enter_context(
            tc.tile_pool(name="ps2", bufs=cfg["ps2"], space="PSUM"))
    else:  # shared single pool sized by ps2
        ps1pool = ps2pool = ctx.enter_context(
            tc.tile_pool(name="ps", bufs=cfg["ps2"], space="PSUM"))

    xeng = nc.scalar if cfg.get("xq") == "scalar" else nc.sync
    b1_sbs = []
    for (seg, yT, xgT, w1, b1, w2, C, hcnt) in segments:
        b1_sb = singles.tile([P, hcnt], mybir.dt.float32, name=f"b1_{seg}")
        xeng.dma_start(b1_sb, b1)
        b1_sbs.append(b1_sb)

    wz = None
    if cfg.get("warmup"):
        in_dt = _DT[cfg["mode"]]
        wz = singles.tile([P, P], in_dt, name="warmz")
        nc.vector.memset(wz, 0.0)

    def run_all(u=0):
        # u>0 reuses the same tile names: buffer-reuse serialization matches
        # the For_i iteration boundary, without the loop-control overhead
        if wz is not None:
            # dependency-free dummy matmuls fill the x/w1 DMA start gap so
            # the PE HAM stays un-throttled when the real stream begins
            tagw = "ps1" if cfg["ps1"] else "ps"
            warm = ps1pool.tile([P, 64], mybir.dt.float32, tag=tagw,
                                name="warmps")
            for _ in range(cfg["warmup"]):
                nc.tensor.matmul(warm, wz, wz[:, :64], start=True, stop=True)
        for i, (seg, yT, xgT, w1, b1, w2, C, hcnt) in enumerate(segments):
            _do_passes(tc, C, cfg, b1_sbs[i], xgT, w1, w2, yT,
                       xpool, w1pool, w2pool, hpool, ypool, ps1pool, ps2pool,
                       hcnt=hcnt, seg=seg, first_seg=(i == 0 and u == 0))

    if reps == 1:
        run_all()
    else:
        with tc.For_i(0, reps, 1):
            for u in range(cfg.get("unroll", 1)):
                run_all(u)


def _do_passes(tc, C, cfg, b1_sb, xgT, w1, w2, yT,
               xpool, w1pool, w2pool, hpool, ypool, ps1pool, ps2pool,
               hcnt=HC, seg="s", first_seg=True):
    nc = tc.nc
    in_dt = _DT[cfg["mode"]]
    h_dt = in_dt
    chunk = cfg["chunk"] if cfg["mode"] == "bf16" else min(cfg["chunk"], 512)
    xeng = nc.scalar if cfg.get("xq") == "scalar" else nc.sync

    # host pre-arranges weights into SBUF tile layout:
    #   w1: [P, hcnt, KC, 128]  (w1_dev[p, hc, kc, h] = w1[kc*128+p, hc*128+h])
    #   w2: [P, DC, hcnt, 128]  (w2_dev[p, dc, hc, d] = w2[hc*128+p, dc*128+d])
    # so each per-tile DMA reads 2-8KB contiguous per partition line.
    xgT_r = xgT.rearrange("(kc p) c -> p kc c", p=P)
    yT_r = yT.rearrange("(dc p) c -> dc p c", p=P)

    chunk_outer = _chunk_outer(cfg)

    first_pass = first_seg
    for (poff, pw) in _passes(C, cfg):
        chunks = _chunks(pw, chunk)
        if cfg.get("maxchunks"):
            chunks = chunks[:cfg["maxchunks"]]
        xg = xpool.tile([P, KC, pw], in_dt, name=f"xg_{seg}")
        w1t0 = None
        kh = KC // 2
        if first_pass and cfg.get("xsplit", 0):
            # critical start path, in consumption order: w1[hc0] tile, then
            # x chunk0 in two kc-halves (the first 4 matmuls unblock after
            # the first half). Later chunks are issued after the w1 DMA
            # stream (not needed until their turn in the chunk-outer order).
            first_pass = False
            with tc.high_priority():
                w1t0 = w1pool.tile([P, KC, P], in_dt,
                                   name=f"w1t_{seg}_{poff}_0")
                nc.sync.dma_start(w1t0, w1[:, 0, :, :])
                (o0, cw0) = chunks[0]
                xeng.dma_start(xg[:, :kh, o0:o0 + cw0],
                               xgT_r[:, :kh, poff + o0:poff + o0 + cw0])
                xeng.dma_start(xg[:, kh:, o0:o0 + cw0],
                               xgT_r[:, kh:, poff + o0:poff + o0 + cw0])
            xrest = chunks[1:]
        elif cfg.get("xsplit", 0):
            xrest = chunks
        else:
            xeng.dma_start(xg, xgT_r[:, :, poff:poff + pw])
            xrest = []
        h = hpool.tile([P, hcnt, pw], h_dt, name=f"h_{seg}")
        tag1 = "ps1" if ps1pool is not ps2pool else "ps"
        tag2 = "ps2" if ps1pool is not ps2pool else "ps"

        if chunk_outer:
            # phase 1, chunk-outer: all w1 tiles stay resident; x chunk c
            # is consumed by hcnt*KC matmuls before chunk c+1 is touched, so
            # only (w1[hc0], x chunk0) gate the start
            w1ts = [w1t0] if w1t0 is not None else []
            for hc in range(len(w1ts), hcnt):
                w1t = w1pool.tile([P, KC, P], in_dt,
                                  name=f"w1t_{seg}_{poff}_{hc}")
                nc.sync.dma_start(w1t, w1[:, hc, :, :])
                w1ts.append(w1t)
            for (o, cw) in xrest:
                xeng.dma_start(xg[:, :kh, o:o + cw],
                               xgT_r[:, :kh, poff + o:poff + o + cw])
                xeng.dma_start(xg[:, kh:, o:o + cw],
                               xgT_r[:, kh:, poff + o:poff + o + cw])
            ci = 0
            while ci < len(chunks):
                if cfg.get("p1reuse") and ci > 0:
                    # remaining chunks together, kc-outer: one stationary
                    # load serves len(rest) matmuls (fewer LDWEIGHTS)
                    rest = chunks[ci:]
                    for hc in range(hcnt):
                        pss = [ps1pool.tile([P, chunk], mybir.dt.float32,
                                            tag=tag1, name=f"ps1r_{cj}")
                               for cj in range(len(rest))]
                        for kc in range(KC):
                            for cj, (o, cw) in enumerate(rest):
                                nc.tensor.matmul(
                                    pss[cj][:, :cw], w1ts[hc][:, kc, :],
                                    xg[:, kc, o:o + cw],
                                    start=(kc == 0), stop=(kc == KC - 1))
                        for cj, (o, cw) in enumerate(rest):
                            nc.scalar.activation(
                                h[:, hc, o:o + cw], pss[cj][:, :cw],
                                mybir.ActivationFunctionType.Gelu,
                                bias=b1_sb[:, hc:hc + 1], scale=1.0)
                    break
                (o, cw) = chunks[ci]
                for hc in range(hcnt):
                    ps = ps1pool.tile([P, chunk], mybir.dt.float32, tag=tag1)
                    for kc in range(KC):
                        nc.tensor.matmul(
                            ps[:, :cw], w1ts[hc][:, kc, :], xg[:, kc, o:o + cw],
                            start=(kc == 0), stop=(kc == KC - 1))
                    if not cfg.get("skip_act"):
                        nc.scalar.activation(
                            h[:, hc, o:o + cw], ps[:, :cw],
                            mybir.ActivationFunctionType.Gelu,
                            bias=b1_sb[:, hc:hc + 1], scale=1.0)
                ci += 1
            if cfg.get("skip_p2"):
                yt = ypool.tile([P, 8], mybir.dt.float32, tag="yt",
                                name=f"ytp1_{seg}")
                src = xg[:, 0, 0:8] if cfg.get("skip_act") else h[:, 0, 0:8]
                nc.vector.tensor_copy(yt, src)
                xeng.dma_start(yT_r[0, :, poff:poff + 8], yt)
            else:
                _phase2(tc, cfg, chunks, chunk, h, w2, yT_r, poff,
                        w2pool, ypool, ps2pool, tag2, in_dt, xeng,
                        hcnt=hcnt, seg=seg)
            continue

        for (o, cw) in xrest:
            xeng.dma_start(xg[:, :, o:o + cw],
                           xgT_r[:, :, poff + o:poff + o + cw])

        # phase 1: h = gelu(x @ w1 + b1), computed as hT[hc, t] chunks
        for hc in range(HC):
            if hc == 0 and w1t0 is not None:
                w1t = w1t0
            else:
                w1t = w1pool.tile([P, KC, P], in_dt)
                nc.sync.dma_start(w1t, w1[:, hc, :, :])
            if cfg["wreuse"]:
                # kc-outer: one stationary load serves every token chunk
                pss = [ps1pool.tile([P, chunk], mybir.dt.float32, tag=tag1,
                                    name=f"ps1_{hc}_{ci}")
                       for ci in range(len(chunks))]
                for kc in range(KC):
                    for ci, (o, cw) in enumerate(chunks):
                        nc.tensor.matmul(
                            pss[ci][:, :cw], w1t[:, kc, :], xg[:, kc, o:o + cw],
                            start=(kc == 0), stop=(kc == KC - 1))
                for ci, (o, cw) in enumerate(chunks):
                    nc.scalar.activation(
                        h[:, hc, o:o + cw], pss[ci][:, :cw],
                        mybir.ActivationFunctionType.Gelu,
                        bias=b1_sb[:, hc:hc + 1], scale=1.0)
            else:
                for (o, cw) in chunks:
                    ps = ps1pool.tile([P, chunk], mybir.dt.float32, tag=tag1)
                    for kc in range(KC):
                        nc.tensor.matmul(
                            ps[:, :cw], w1t[:, kc, :], xg[:, kc, o:o + cw],
                            start=(kc == 0), stop=(kc == KC - 1))
                    nc.scalar.activation(
                        h[:, hc, o:o + cw], ps[:, :cw],
                        mybir.ActivationFunctionType.Gelu,
                        bias=b1_sb[:, hc:hc + 1], scale=1.0)

        # phase 2: yT[dc, t] = (h.T @ w2) chunks
        for dc in range(DC):
            halves = None
            if cfg.get("w2split") and not cfg["wreuse"]:
                # half-size w2 tiles free 8KB/partition of SBUF (lets the
                # whole hidden activation block fit in a single pass)
                hh = HC // 2
                w2ta = w2pool.tile([P, hh, P], in_dt, tag="w2t",
                                   name=f"w2a_{poff}_{dc}")
                w2eng.dma_start(w2ta, w2[:, dc, :hh, :])
                w2tb = w2pool.tile([P, hh, P], in_dt, tag="w2t",
                                   name=f"w2b_{poff}_{dc}")
                w2eng.dma_start(w2tb, w2[:, dc, hh:, :])
                halves = [(w2ta, 0), (w2tb, hh)]
                w2t = None
            else:
                w2t = w2pool.tile([P, HC, P], in_dt, tag="w2t",
                                  name=f"w2t_{poff}_{dc}")
                nc.sync.dma_start(w2t, w2[:, dc, :, :])
            if cfg["wreuse"]:
                pss = [ps2pool.tile([P, chunk], mybir.dt.float32, tag=tag2,
                                    name=f"ps2_{dc}_{ci}")
                       for ci in range(len(chunks))]
                for hc in range(HC):
                    for ci, (o, cw) in enumerate(chunks):
                        nc.tensor.matmul(
                            pss[ci][:, :cw], w2t[:, hc, :], h[:, hc, o:o + cw],
                            start=(hc == 0), stop=(hc == HC - 1))
                for ci, (o, cw) in enumerate(chunks):
                    yt = ypool.tile([P, cw], mybir.dt.float32, tag="yt",
                                    name=f"yt_{dc}_{ci}")
                    nc.vector.tensor_copy(yt[:, :cw], pss[ci][:, :cw])
                    xeng.dma_start(
                        yT_r[dc, :, poff + o:poff + o + cw], yt[:, :cw])
            else:
                dchunks = chunks
                if dc == DC - 1 and cfg.get("tailsplit", True):
                    # halve the final chunk so its copy+DMA-out overlaps the
                    # matmuls of the other half (shortens the drain tail)
                    (o_l, cw_l) = chunks[-1]
                    ch = -(-(cw_l // 2) // 8) * 8
                    dchunks = list(chunks[:-1]) + [(o_l, ch),
                                                   (o_l + ch, cw_l - ch)]
                for (o, cw) in dchunks:
                    ps2 = ps2pool.tile([P, chunk], mybir.dt.float32, tag=tag2)
                    for hc in range(HC):
                        if halves is None:
                            lhsT = w2t[:, hc, :]
                        else:
                            wt, base = halves[hc // (HC // 2)]
                            lhsT = wt[:, hc - base, :]
                        nc.tensor.matmul(
                            ps2[:, :cw], lhsT, h[:, hc, o:o + cw],
                            start=(hc == 0), stop=(hc == HC - 1))
                    yt = ypool.tile([P, cw], mybir.dt.float32, tag="yt",
                                    name=f"yt_{dc}_{o}")
                    nc.vector.tensor_copy(yt[:, :cw], ps2[:, :cw])
                    xeng.dma_start(
                        yT_r[dc, :, poff + o:poff + o + cw], yt[:, :cw])


def _phase2(tc, cfg, chunks, chunk, h, w2, yT_r, poff,
            w2pool, ypool, ps2pool, tag2, in_dt, xeng, hcnt=HC, seg="s"):
    nc = tc.nc
    y_dt = in_dt if cfg.get("ybf16") else mybir.dt.float32
    w2eng = xeng if cfg.get("w2q") == "scalar" else nc.sync
    for dc in range(DC):
        halves = None
        if cfg.get("w2split"):
            hh = hcnt // 2
            w2ta = w2pool.tile([P, hh, P], in_dt, tag="w2t",
                               name=f"w2a_{seg}_{poff}_{dc}")
            w2eng.dma_start(w2ta, w2[:, dc, :hh, :])
            w2tb = w2pool.tile([P, hh, P], in_dt, tag="w2t",
                               name=f"w2b_{seg}_{poff}_{dc}")
            w2eng.dma_start(w2tb, w2[:, dc, hh:, :])
            halves = [(w2ta, 0), (w2tb, hh)]
            w2t = None
        else:
            w2t = w2pool.tile([P, hcnt, P], in_dt, tag="w2t",
                              name=f"w2t_{seg}_{poff}_{dc}")
            w2eng.dma_start(w2t, w2[:, dc, :, :])
        dchunks = chunks
        if dc == DC - 1 and cfg.get("tailsplit", True):
            (o_l, cw_l) = chunks[-1]
            ch = -(-(cw_l // 2) // 8) * 8
            dchunks = list(chunks[:-1]) + [(o_l, ch), (o_l + ch, cw_l - ch)]

        def lhs(hc):
            if halves is None:
                return w2t[:, hc, :]
            wt, base = halves[hc // (hcnt // 2)]
            return wt[:, hc - base, :]

        if cfg.get("p2reuse") and dc < DC - 1:
            # hc-outer: one stationary load serves every token chunk
            pss = [ps2pool.tile([P, chunk], mybir.dt.float32, tag=tag2,
                                name=f"ps2r_{cj}")
                   for cj in range(len(dchunks))]
            for hc in range(hcnt):
                for cj, (o, cw) in enumerate(dchunks):
                    nc.tensor.matmul(
                        pss[cj][:, :cw], lhs(hc), h[:, hc, o:o + cw],
                        start=(hc == 0), stop=(hc == hcnt - 1))
            for cj, (o, cw) in enumerate(dchunks):
                yt = ypool.tile([P, cw], y_dt, tag="yt",
                                name=f"yt_{seg}_{dc}_{o}")
                nc.vector.tensor_copy(yt[:, :cw], pss[cj][:, :cw])
                xeng.dma_start(
                    yT_r[dc, :, poff + o:poff + o + cw], yt[:, :cw])
            continue
        for (o, cw) in dchunks:
            ps2 = ps2pool.tile([P, chunk], mybir.dt.float32, tag=tag2)
            for hc in range(hcnt):
                nc.tensor.matmul(
                    ps2[:, :cw], lhs(hc), h[:, hc, o:o + cw],
                    start=(hc == 0), stop=(hc == hcnt - 1))
            yt = ypool.tile([P, cw], y_dt, tag="yt",
                            name=f"yt_{seg}_{dc}_{o}")
            nc.vector.tensor_copy(yt[:, :cw], ps2[:, :cw])
            xeng.dma_start(
                yT_r[dc, :, poff + o:poff + o + cw], yt[:, :cw])


def _fold_mm_incs(nc):
    """Drop the per-matmul semaphore increment from non-stop matmuls (only
    the accumulation group's stop-MM keeps its inc) and remap every wait
    threshold on those semaphores to the new counting. MMs complete in PE
    program order, so a waiter on a mid-group count now unblocks at the
    group's stop-MM — always later, never earlier. Saves the ~26ns
    serialized EVT_SEM write per matmul."""
    import bass_rust
    from concourse import mybir as mb

    folded = 0
    for blk in nc.m.functions[0].blocks:
        mm_sems = set()
        for inst in blk.instructions:
            si = inst.sync_info
            if isinstance(inst, mb.InstMatmult) and si:
                for u in si.on_update:
                    if u.sync_type == "semaphore" and \
                            u.update_mode == "sem-inc" and \
                            u.update_reg is None and u.update_value == 1:
                        mm_sems.add(u.id)
        for inst in blk.instructions:
            si = inst.sync_info
            if si is None:
                continue
            if not isinstance(inst, mb.InstMatmult):
                for u in si.on_update:
                    mm_sems.discard(u.id)
            for w in si.on_wait:
                # only immediate GE waits are remappable
                if w.id in mm_sems and w.wait_mode != "sem-ge-imm":
                    mm_sems.discard(w.id)
        if not mm_sems:
            continue

        # per sem: old cumulative position -> new cumulative after remap,
        # where an old threshold T maps to the count of kept incs up to the
        # first kept inc at position >= T
        mms = [i for i in blk.instructions if isinstance(i, mb.InstMatmult)]
        remap = {}
        for sid in mm_sems:
            keeps = []
            for j, inst in enumerate(mms):
                si = inst.sync_info
                if not si or not any(u.id == sid for u in si.on_update):
                    continue
                keeps.append(inst.stop_tensor_calc or inst is mms[-1])
            total_old = len(keeps)
            fwd = [0] * (total_old + 1)  # fwd[T] for T in 1..total
            kept_cum = 0
            pending = []
            for pos in range(1, total_old + 1):
                pending.append(pos)
                if keeps[pos - 1]:
                    kept_cum += 1
                    for p in pending:
                        fwd[p] = kept_cum
                    pending = []
            for p in pending:  # trailing dropped (shouldn't happen)
                fwd[p] = kept_cum
            remap[sid] = fwd

        # bail on sems with waits beyond this block's total (cross-block
        # counting would make the per-block remap unsound)
        for inst in blk.instructions:
            si = inst.sync_info
            if si:
                for w in si.on_wait:
                    if w.id in remap and w.wait_value >= len(remap[w.id]):
                        del remap[w.id]
        mm_sems &= set(remap)
        if not mm_sems:
            continue

        # rewrite updates on MMs
        for sid in mm_sems:
            seen = 0
            for inst in mms:
                si = inst.sync_info
                if not si or not any(u.id == sid for u in si.on_update):
                    continue
                seen += 1
                keep = inst.stop_tensor_calc or inst is mms[-1]
                if not keep:
                    new_upd = [u for u in si.on_update if u.id != sid]
                    inst.sync_info = bass_rust.SyncInfo(
                        on_wait=list(si.on_wait), on_update=new_upd)
                    folded += 1

        # rewrite waits everywhere in the block
        for inst in blk.instructions:
            si = inst.sync_info
            if not si or not si.on_wait:
                continue
            new_w, changed = [], False
            for w in si.on_wait:
                if w.id in mm_sems:
                    fwd = remap[w.id]
                    t = min(max(w.wait_value, 0), len(fwd) - 1)
                    nv = fwd[t] if t > 0 else 0
                    if nv != w.wait_value:
                        w = bass_rust.SyncWait(
                            sync_type=w.sync_type, id=w.id,
                            ant_name=w.ant_name, wait_mode=w.wait_mode,
                            wait_value=nv, wait_reg=None)
                        changed = True
                new_w.append(w)
            if changed:
                inst.sync_info = bass_rust.SyncInfo(
                    on_wait=new_w, on_update=list(si.on_update))
    return folded


def _dedupe_ldweights(nc):
    """Drop an InstLdweights that reloads exactly what the PE already holds
    (same SBUF slot/offset/pattern, no semaphores attached). Safe because the
    wreuse ordering makes duplicates strictly consecutive in PE order."""
    def sig(ap):
        return (ap.memref, ap.offset, str(ap.ap), ap.dtype)

    removed = 0
    for blk in nc.m.functions[0].blocks:
        out = []
        last_sig = None
        changed = False
        for inst in blk.instructions:
            if inst.engine == mybir.EngineType.PE:
                if isinstance(inst, mybir.InstLdweights):
                    s = sig(inst.ins[0])
                    si = inst.sync_info
                    no_sync = (si is None) or (
                        not si.on_wait and not si.on_update)
                    if s == last_sig and no_sync:
                        removed += 1
                        changed = True
                        continue
                    last_sig = s
                elif not isinstance(inst, mybir.InstMatmult):
                    last_sig = None  # drains/branches etc: invalidate
            out.append(inst)
        if changed:
            blk.instructions = out
    return removed


def _build(C, cfg, reps=1):
    key = (C, reps, tuple(sorted(cfg.items())))
    if key in _CACHE:
        return _CACHE[key]
    dt_in = _DT[cfg["mode"]]
    nc = bacc.Bacc("TRN2", target_bir_lowering=False, debug=False,
                   num_devices=N_CORES)
    if cfg.get("halfh"):
        Ca, Cb = C
        hh = HC // 2
        segs = []
        for seg, Cs in (("a", Ca), ("b", Cb)):
            xgT = nc.dram_tensor(f"xgT_{seg}", (D, Cs), dt_in,
                                 kind="ExternalInput").ap()
            w1a = nc.dram_tensor(f"w1_{seg}", (P, hh, KC, P), dt_in,
                                 kind="ExternalInput").ap()
            b1a = nc.dram_tensor(f"b1_{seg}", (P, hh), mybir.dt.float32,
                                 kind="ExternalInput").ap()
            w2a = nc.dram_tensor(f"w2_{seg}", (P, DC, hh, P), dt_in,
                                 kind="ExternalInput").ap()
            y_dt = dt_in if cfg.get("ybf16") else mybir.dt.float32
            yT = nc.dram_tensor(f"yT_{seg}", (D, Cs), y_dt,
                                kind="ExternalOutput").ap()
            segs.append((seg, yT, xgT, w1a, b1a, w2a, Cs, hh))
    else:
        xgT = nc.dram_tensor("xgT", (D, C), dt_in, kind="ExternalInput").ap()
        w1a = nc.dram_tensor("w1", (P, HC, KC, P), dt_in,
                             kind="ExternalInput").ap()
        b1a = nc.dram_tensor("b1", (P, HC), mybir.dt.float32,
                             kind="ExternalInput").ap()
        w2a = nc.dram_tensor("w2", (P, DC, HC, P), dt_in,
                             kind="ExternalInput").ap()
        y_dt = dt_in if cfg.get("ybf16") else mybir.dt.float32
        yT = nc.dram_tensor("yT", (D, C), y_dt,
                            kind="ExternalOutput").ap()
        segs = [("s", yT, xgT, w1a, b1a, w2a, C, HC)]
    with tile.TileContext(nc) as tc:
        _ffn_body(tc, segs, cfg, reps=reps)
    nc.compile()
    if cfg["dedup"]:
        _dedupe_ldweights(nc)
    _CACHE[key] = nc
    return nc


def _w1_dev(w1_e, np_in, hcnt=HC):
    # [D, h] -> [P, hcnt, KC, 128]: w1_dev[p, hc, kc, h] = w1[kc*128+p, hc*128+h]
    return np.ascontiguousarray(
        w1_e.reshape(KC, P, hcnt, P).transpose(1, 2, 0, 3)).astype(np_in)


def _w2_dev(w2_e, np_in, hcnt=HC):
    # [h, D] -> [P, DC, hcnt, 128]: w2_dev[p, dc, hc, d] = w2[hc*128+p, dc*128+d]
    return np.ascontiguousarray(
        w2_e.reshape(hcnt, P, DC, P).transpose(1, 2, 0, 3)).astype(np_in)


def _route(xf, router_w):
    """Replicates the reference router in fp32 numpy: softmax, top-2,
    renormalize. Returns per-expert token ids and combine weights."""
    logits = xf @ np.asarray(router_w, dtype=np.float32)          # [T, E]
    m = logits.max(axis=-1, keepdims=True)
    z = np.exp(logits - m)
    probs = z / z.sum(axis=-1, keepdims=True)
    idx = np.argpartition(-probs, TOP_K - 1, axis=-1)[:, :TOP_K]  # [T, 2]
    vals = np.take_along_axis(probs, idx, axis=-1)
    wn = vals / vals.sum(axis=-1, keepdims=True)

    eflat = idx.reshape(-1)
    tflat = np.repeat(np.arange(T), TOP_K)
    wflat = wn.reshape(-1).astype(np.float32)
    order = np.argsort(eflat, kind="stable")
    counts = np.bincount(eflat, minlength=E)
    starts = np.concatenate([[0], np.cumsum(counts)])
    toks, wts = [], []
    for e in range(E):
        sel = order[starts[e]:starts[e + 1]]
        toks.append(tflat[sel])
        wts.append(wflat[sel])
    return toks, wts, counts


def _capacity(counts, cfg):
    g = cfg["cgran"]
    if cfg.get("halfh"):
        pairs = _pairing(counts)

        def cap(cs):
            return max(cfg["chunk"],
                       int(math.ceil(max(cs) / g)) * g)
        return (cap([counts[a] for a, _ in pairs]),
                cap([counts[b] for _, b in pairs]))
    return max(cfg["chunk"], int(math.ceil(counts.max() / g)) * g)


def _pairing(counts):
    # hot expert paired with cold: core pair j serves experts pairs[j];
    # core 2j holds the lower H-half of both, core 2j+1 the upper half
    order = np.argsort(-np.asarray(counts), kind="stable")
    return [(int(order[i]), int(order[E - 1 - i])) for i in range(E // 2)]


def _in_maps(xf, toks, counts, C, w1, b1, w2, cfg):
    np_in = _NPDT[cfg["mode"]]
    if cfg.get("halfh"):
        Ca, Cb = C
        hh = HC // 2
        pairs = _pairing(counts)
        maps = []
        for (ea, eb) in pairs:
            xga = np.zeros((D, Ca), dtype=np_in)
            xga[:, :counts[ea]] = xf[toks[ea]].T.astype(np_in)
            xgb = np.zeros((D, Cb), dtype=np_in)
            xgb[:, :counts[eb]] = xf[toks[eb]].T.astype(np_in)
            for half in range(2):
                sl = slice(half * (H // 2), (half + 1) * (H // 2))
                maps.append({
                    "xgT_a": xga, "xgT_b": xgb,
                    "w1_a": _w1_dev(w1[ea][:, sl], np_in, hh),
                    "w1_b": _w1_dev(w1[eb][:, sl], np_in, hh),
                    "b1_a": np.ascontiguousarray(
                        b1[ea][sl].reshape(hh, P).T),
                    "b1_b": np.ascontiguousarray(
                        b1[eb][sl].reshape(hh, P).T),
                    "w2_a": _w2_dev(w2[ea][sl, :], np_in, hh),
                    "w2_b": _w2_dev(w2[eb][sl, :], np_in, hh),
                })
        return maps
    maps = []
    for e in range(E):
        ce = counts[e]
        xg = np.zeros((D, C), dtype=np_in)
        xg[:, :ce] = xf[toks[e]].T.astype(np_in)
        maps.append({
            "xgT": xg,
            "w1": _w1_dev(w1[e], np_in),
            "b1": np.ascontiguousarray(b1[e].reshape(HC, P).T),
            "w2": _w2_dev(w2[e], np_in),
        })
    return maps


def kernel(x, router_w, w1, b1, w2, b2):
    global LAST_RESULTS
    x = np.asarray(x, dtype=np.float32)
    w1 = np.asarray(w1, dtype=np.float32)
    b1 = np.asarray(b1, dtype=np.float32)
    w2 = np.asarray(w2, dtype=np.float32)
    b2 = np.asarray(b2, dtype=np.float32)
    cfg = dict(DEFAULT_CFG)

    xf = x.reshape(T, D)
    toks, wts, counts = _route(xf, router_w)
    C = _capacity(counts, cfg)
    if not cfg.get("halfh") and C > 1100 and cfg["mode"] != "bf16":
        # single-pass fp32 h block no longer fits in SBUF; fall back to two
        # overlapped passes (correct, slightly slower)
        cfg["passw"] = (C + 1) // 2
        cfg["hb"] = 2 if C <= 1152 else 1

    nc = _build(C, cfg)
    in_maps = _in_maps(xf, toks, counts, C, w1, b1, w2, cfg)

    global LDWOPT_ACTIVE
    LDWOPT_ACTIVE = bool(cfg["ldwopt"])
    if LDWOPT_ACTIVE:
        _install_ldwopt_patch()
    res = run_bass_kernel_spmd(nc, in_maps, core_ids=list(range(N_CORES)))
    LAST_RESULTS = res

    out = np.zeros((T, D), dtype=np.float32)
    if cfg.get("halfh"):
        for j, (ea, eb) in enumerate(_pairing(counts)):
            r0, r1 = res.results[2 * j], res.results[2 * j + 1]
            for seg, e in (("a", ea), ("b", eb)):
                ce = counts[e]
                if ce == 0:
                    continue
                y = (r0[f"yT_{seg}"][:, :ce].astype(np.float32)
                     + r1[f"yT_{seg}"][:, :ce].astype(np.float32)).T
                out[toks[e]] += wts[e][:, None] * (y + b2[e][None, :])
        return out.reshape(B, S, D)
    for e in range(E):
        ce = counts[e]
        if ce == 0:
            continue
        y = res.results[e]["yT"][:, :ce].astype(np.float32).T  # [ce, D]
        out[toks[e]] += wts[e][:, None] * (y + b2[e][None, :])
    return out.reshape(B, S, D)

